# revision 1
# baseline (speedup 1.0000x reference)
"""Trainium2 Bass kernel for CombinedGCN (2x GCNConv + mean-pool + 2 FC).

Sharding: core k owns dst nodes [50000k, 50000(k+1)) == graph k (data parallel).

Math factorization (PyG GCNConv with self-loops, sym norm):
  out_i = dis_i * ( sum_{real edges e->i} dis_src * h_src  +  dis_i * h_i ) + b
with dis = 1/sqrt(deg incl self-loop).  All per-edge weights become per-row
scalings; aggregation is an unweighted gather-sum over real edges plus a
purely local self term.

Device pipeline per core:
  conv1: host stages the gathered+scaled edge stream (the core's edge shard)
         in a degree-bucketed segment layout -> sequential DMA, strided
         segmented sum on DVE, + self term, scale, matmuls W1/W2 on PE.
         Output h2~ = dis*h2 written in pi-row layout, AllGathered in 4
         row-chunks (overlapped with compute).
  conv2: sources are device-produced -> two-stage windowed dma_gather
         (int16 indices, 32k-row windows): stage 1 gathers bucket-major by
         source window into an HBM staging buffer; stage 2 gathers from
         staging (<32k rows) into the segment layout.  Then segmented sum,
         + self term (local), scale, bias, relu, mean-pool, FC head.
"""
import sys

import numpy as np

sys.path.insert(0, "/opt/trn_rl_repo")

from concourse import bass, bacc, mybir, tile  # noqa: E402
from concourse.masks import make_identity  # noqa: E402

B = 8
F = 64
H1 = 128
EMB = 64
P = 128
F32 = mybir.dt.float32
I16 = mybir.dt.int16
WIN = 32768           # int16 gather window (rows)
SUBCALL = 8192        # max slots per dma_gather call
RUN_MAX_BLOCKS = 230  # stage-2 staging payload blocks per run
NCHUNK = 6            # AllGather chunks


def _wrap_idx16(flat):
    """[num] int16 (num % 16 == 0) -> [128, num//16] wrapped + replicated."""
    num = len(flat)
    s = flat.reshape(num // 16, 16).T           # [16, num//16]
    return np.tile(s, (8, 1)).astype(np.int16)  # [128, num//16]


def _plan(c_all, n_per):
    """Common cross-core schedule from real-edge counts c_all [B*n_per]."""
    G = n_per // P + 1
    R = G * P
    orders, invs = [], []
    Cg = np.zeros(G, np.int64)
    for k in range(B):
        ck = c_all[k * n_per:(k + 1) * n_per]
        order = np.lexsort((np.arange(n_per), -ck))   # c desc, node asc
        inv = np.empty(n_per, np.int64)
        inv[order] = np.arange(n_per)
        orders.append(order)
        invs.append(inv)
        cpad = np.zeros(R, np.int64)
        cpad[:n_per] = ck[order]
        Cg = np.maximum(Cg, cpad.reshape(G, P).max(axis=1))
    batches = []  # (g0, NB, Cb)
    g = 0
    while g < G:
        Cb = int(Cg[g])
        NB = 1
        while NB < 4 and g + NB < G and (NB + 1) * max(Cb, 1) <= 32:
            NB += 1
        if NB == 3:
            NB = 2
        batches.append((g, NB, Cb))
        g += NB
    bofs1 = [0]          # conv1 grid: Cb+1 slots per node (last = self term)
    for (_, NB, Cb) in batches:
        bofs1.append(bofs1[-1] + P * NB * (Cb + 1))
    bofs2 = [0]          # conv2 grid: Cb slots per node
    for (_, NB, Cb) in batches:
        bofs2.append(bofs2[-1] + P * NB * Cb)
    # AllGather chunks: split batches into NCHUNK spans of ~equal groups
    chunks = []   # (batch_lo, batch_hi, g_lo, g_hi)
    bi = 0
    for ci in range(NCHUNK):
        target = (G * (ci + 1) + NCHUNK - 1) // NCHUNK
        lo = bi
        if ci == NCHUNK - 1:
            bi = len(batches)
        else:
            while bi < len(batches) and batches[bi][0] + batches[bi][1] <= target:
                bi += 1
        g_lo = batches[lo][0]
        g_hi = batches[bi - 1][0] + batches[bi - 1][1] if bi > lo else g_lo
        if bi > lo:
            chunks.append((lo, bi, g_lo, g_hi))
    assert chunks[-1][3] == G
    # stage-2 runs: consecutive batches, sum of W blocks <= RUN_MAX_BLOCKS
    runs = []     # (batch_lo, batch_hi)
    bi = 0
    while bi < len(batches):
        lo = bi
        blocks = 0
        while bi < len(batches):
            w = batches[bi][1] * batches[bi][2]
            if blocks + w > RUN_MAX_BLOCKS and bi > lo:
                break
            blocks += w
            bi += 1
        runs.append((lo, bi))
    s2groups = []   # (batch_lo, batch_hi, Wsum) within one run, Wsum <= 32
    for (rlo, rhi) in runs:
        bi2 = rlo
        while bi2 < rhi:
            lo2 = bi2
            wsum = 0
            while bi2 < rhi:
                w = batches[bi2][1] * batches[bi2][2]
                if wsum + w > 32 and bi2 > lo2:
                    break
                wsum += w
                bi2 += 1
            s2groups.append((lo2, bi2, wsum))
    return orders, invs, Cg, batches, bofs1, bofs2, chunks, runs, s2groups, G, R


def _preprocess(inputs):
    nf = np.ascontiguousarray(np.asarray(inputs["node_features"], np.float32))
    ei = np.asarray(inputs["edge_index"]).reshape(2, -1)
    _b, n_per, _f = nf.shape
    assert _b == B and _f == F
    x = nf.reshape(-1, F)
    N = x.shape[0]
    src = ei[0].astype(np.int64)
    dst = ei[1].astype(np.int64)
    creal = np.bincount(dst, minlength=N)          # real in-degree
    deg = creal + 1                                 # incl self-loop
    dis = 1.0 / np.sqrt(deg.astype(np.float64))
    (orders, invs, Cg, batches, bofs1, bofs2, chunks, runs, s2groups,
     G, R) = _plan(creal, n_per)
    S1 = bofs1[-1]
    S2 = bofs2[-1]
    nbat = len(batches)

    eo = np.argsort(dst, kind="stable")
    s_s = src[eo]
    d_s = dst[eo]
    starts = np.zeros(N + 1, np.int64)
    starts[1:] = np.cumsum(creal)

    g2b = np.zeros(G, np.int64)
    g2gl = np.zeros(G, np.int64)
    for bi, (g0, NB, Cb) in enumerate(batches):
        g2b[g0:g0 + NB] = bi
        g2gl[g0:g0 + NB] = np.arange(NB)
    Cb_arr = np.array([b[2] for b in batches])
    W_arr = np.array([b[1] * b[2] for b in batches])
    W1_arr = np.array([b[1] * (b[2] + 1) for b in batches])
    bofs1_arr = np.array(bofs1[:-1])
    bofs2_arr = np.array(bofs2[:-1])
    run_of_batch = np.zeros(nbat, np.int64)
    for ri, (lo, hi) in enumerate(runs):
        run_of_batch[lo:hi] = ri
    nchunk = len(chunks)
    chunk_of_group = np.zeros(G, np.int64)
    cstart_rows = np.zeros(nchunk, np.int64)
    crows = np.zeros(nchunk, np.int64)
    for ci, (blo, bhi, g_lo, g_hi) in enumerate(chunks):
        chunk_of_group[g_lo:g_hi] = ci
        cstart_rows[ci] = g_lo * P
        crows[ci] = (g_hi - g_lo) * P

    inv_all = np.concatenate(invs)
    NRTOT = RUN_MAX_BLOCKS + 26

    w1e = np.concatenate([np.asarray(inputs["W1"], np.float32),
                          np.asarray(inputs["b1"], np.float32)[None, :]], axis=0)
    w2 = np.ascontiguousarray(np.asarray(inputs["W2"], np.float32))
    fce = np.concatenate([np.asarray(inputs["fc_w"], np.float32),
                          np.asarray(inputs["fc_b"], np.float32)[None, :]], axis=0)
    oute = np.concatenate([np.asarray(inputs["out_w"], np.float32),
                           np.asarray(inputs["out_b"], np.float32)[None, :]], axis=0)
    b2b = np.tile(np.asarray(inputs["b2"], np.float32)[None, :], (P, 4)).astype(np.float32)
    pmask = (np.arange(P) + (G - 1) * P < n_per).astype(np.float32)[:, None].copy()

    in_maps = []
    common_calls = None
    i1_total = 0
    group_subs = {}
    for k in range(B):
        lo = k * n_per
        order = orders[k]
        inv = invs[k]
        e0, e1 = starts[lo], starts[lo + n_per]
        es = s_s[e0:e1]
        ed = d_s[e0:e1]
        j_e = np.arange(e0, e1) - starts[ed]
        q = inv[ed - lo]
        ge = q // P
        pe = q % P
        bi_e = g2b[ge]
        pos1 = (bofs1_arr[bi_e] + pe * W1_arr[bi_e]
                + g2gl[ge] * (Cb_arr[bi_e] + 1) + j_e)

        g1 = np.zeros((S1, F), np.float32)
        g1[pos1] = (x[es] * dis[es][:, None]).astype(np.float32)
        # self slot: node q (pi order) at slot index c_q
        qq = np.arange(n_per)
        q_ge = invs[k][qq] // P
        q_pe = invs[k][qq] % P
        q_bi = g2b[q_ge]
        cq = creal[lo + qq]
        spos = (bofs1_arr[q_bi] + q_pe * W1_arr[q_bi]
                + g2gl[q_ge] * (Cb_arr[q_bi] + 1) + cq)
        g1[spos] = (x[lo + qq] * dis[lo + qq][:, None]).astype(np.float32)

        b_src = es // n_per
        q_src = inv_all[es]
        c_src = chunk_of_group[q_src // P]
        row_in = b_src * crows[c_src] + (q_src - cstart_rows[c_src])

        run_e = run_of_batch[bi_e]
        win_e = row_in // WIN
        if common_calls is None:
            # first core: collect per-(run,chunk,window) counts for all cores
            # to build the COMMON call schedule (same NEFF on every core)
            counts = {}
            for kk in range(B):
                e0k, e1k = starts[kk * n_per], starts[(kk + 1) * n_per]
                esk = s_s[e0k:e1k]
                edk = d_s[e0k:e1k]
                qk = invs[kk][edk - kk * n_per]
                gek = qk // P
                bik = g2b[gek]
                qsk = inv_all[esk]
                csk = chunk_of_group[qsk // P]
                rik = b_src_rows = esk // n_per
                rowk = rik * crows[csk] + (qsk - cstart_rows[csk])
                runk = run_of_batch[bik]
                wink = rowk // WIN
                key = runk * 1000 + csk * 100 + wink
                u, cnt = np.unique(key, return_counts=True)
                for kv, cv in zip(u, cnt):
                    counts[int(kv)] = max(counts.get(int(kv), 0), int(cv))
            common_calls = [[] for _ in runs]
            i1_total = 0
            for kv in sorted(counts):
                ri_ = kv // 1000
                ci_ = (kv // 100) % 10
                wi_ = kv % 100
                mtot = counts[kv]
                for off in range(0, mtot, SUBCALL):
                    m = min(SUBCALL, mtot - off)
                    nblk = (m + P - 1) // P
                    common_calls[ri_].append(
                        (ci_, wi_, nblk, nblk * P, i1_total))
                    i1_total += nblk * P // 16 * P
            # per-run staging block offsets
            call_nofs = [[] for _ in runs]
            for ri_ in range(len(runs)):
                nofs = 1
                for (ci_, wi_, nblk, num, i1o) in common_calls[ri_]:
                    call_nofs[ri_].append(nofs)
                    nofs += nblk
                assert nofs <= NRTOT, (nofs, NRTOT)
            # map (ri,ci,wi) -> list of (sub_start, nblk, nofs, i1o)
            group_subs = {}
            for ri_ in range(len(runs)):
                cum = {}
                for (ci_, wi_, nblk, num, i1o), nofs in zip(
                        common_calls[ri_], call_nofs[ri_]):
                    gkey = (ri_, ci_, wi_)
                    group_subs.setdefault(gkey, []).append((nblk, nofs, i1o))
        # per-core: place slots into the common grid
        okey = run_e * 1000 + c_src * 100 + win_e
        so = np.lexsort((np.arange(len(es)), okey))
        stg_row = np.empty(len(es), np.int64)
        i1 = np.zeros(i1_total, np.int16)
        sel_sorted = so
        key_sorted = okey[so]
        cut = np.flatnonzero(np.diff(key_sorted)) + 1
        groups = np.split(sel_sorted, cut) if len(sel_sorted) else []
        for grp in groups:
            kv = int(okey[grp[0]])
            ri_ = kv // 1000
            ci_ = (kv // 100) % 10
            wi_ = kv % 100
            subs = group_subs[(ri_, ci_, wi_)]
            for si, (nblk, nofs, i1o) in enumerate(subs):
                sub = grp[si * SUBCALL:(si + 1) * SUBCALL]
                num = nblk * P
                flat = np.zeros(num, np.int16)
                m = len(sub)
                if m:
                    flat[:m] = (row_in[sub] - wi_ * WIN).astype(np.int16)
                    l = np.arange(m)
                    stg_row[sub] = (l % P) * NRTOT + nofs + l // P
                i1[i1o:i1o + num // 16 * P] = _wrap_idx16(flat).reshape(-1)

        i2_flat = np.zeros(max(S2, 1), np.int64)
        pos2_local = (g2gl[ge] * Cb_arr[bi_e] + j_e) * P + pe
        i2_flat[bofs2_arr[bi_e] + pos2_local] = stg_row
        i2_parts = []
        for (blo2, bhi2, wsum) in s2groups:
            num = P * wsum
            if num == 0:
                continue
            o0 = bofs2[blo2]
            i2_parts.append(_wrap_idx16(
                i2_flat[o0:o0 + num].astype(np.int16)))
        i2 = (np.concatenate([p.reshape(-1) for p in i2_parts])
              if i2_parts else np.zeros(16, np.int16))

        degp = np.ones(R, np.float32)
        degp[:n_per] = deg[lo:lo + n_per][order]

        in_maps.append({
            "g1": np.ascontiguousarray(g1.reshape(-1)),
            "i1": i1.astype(np.int16),
            "i2": i2.astype(np.int16),
            "degp": np.ascontiguousarray(degp.reshape(G, P).T),
            "w1e": w1e, "w2": w2, "fce": fce, "oute": oute,
            "b2b": b2b, "pmask": pmask,
        })
    maxlen1 = max(len(m["i1"]) for m in in_maps)
    maxlen2 = max(len(m["i2"]) for m in in_maps)
    for m in in_maps:
        m["i1"] = np.pad(m["i1"], (0, maxlen1 - len(m["i1"])))
        m["i2"] = np.pad(m["i2"], (0, maxlen2 - len(m["i2"])))
    plan = dict(batches=batches, bofs1=bofs1, chunks=chunks, runs=runs,
                s2groups=s2groups,
                G=G, R=R, S1=S1, n_per=n_per, calls=common_calls,
                NRTOT=NRTOT, crows=[int(c) for c in crows],
                i1_len=maxlen1, i2_len=maxlen2)
    return in_maps, plan


def _segsum(nc, Tv, Cb):
    """Fold [P, NB, Cb, F] into block 0 along axis 2."""
    cc = Cb
    h = 1 << (cc.bit_length() - 1)
    if h < cc:
        nc.vector.tensor_tensor(out=Tv[:, :, 0:cc - h, :], in0=Tv[:, :, 0:cc - h, :],
                                in1=Tv[:, :, h:cc, :], op=mybir.AluOpType.add)
    cc = h
    while cc > 1:
        cc //= 2
        nc.vector.tensor_tensor(out=Tv[:, :, 0:cc, :], in0=Tv[:, :, 0:cc, :],
                                in1=Tv[:, :, cc:2 * cc, :], op=mybir.AluOpType.add)


def _build(plan):
    batches = plan["batches"]
    bofs1 = plan["bofs1"]
    chunks = plan["chunks"]
    runs = plan["runs"]
    s2groups = plan["s2groups"]
    calls = plan["calls"]
    G, R, S1, n_per = plan["G"], plan["R"], plan["S1"], plan["n_per"]
    NRTOT = plan["NRTOT"]
    crows = plan["crows"]

    nc = bacc.Bacc("TRN2", target_bir_lowering=False, debug=False, num_devices=B)
    g1_in = nc.declare_dram_parameter("g1", [S1 * F], F32, isOutput=False)
    i1_in = nc.declare_dram_parameter("i1", [max(plan["i1_len"], 16)], I16, isOutput=False)
    i2_in = nc.declare_dram_parameter("i2", [max(plan["i2_len"], 16)], I16, isOutput=False)
    degp_in = nc.declare_dram_parameter("degp", [P, G], F32, isOutput=False)
    w1e_in = nc.declare_dram_parameter("w1e", [F + 1, H1], F32, isOutput=False)
    w2_in = nc.declare_dram_parameter("w2", [H1, EMB], F32, isOutput=False)
    fce_in = nc.declare_dram_parameter("fce", [EMB + 1, EMB], F32, isOutput=False)
    oute_in = nc.declare_dram_parameter("oute", [EMB + 1, EMB], F32, isOutput=False)
    b2b_in = nc.declare_dram_parameter("b2b", [P, 4 * EMB], F32, isOutput=False)
    pmask_in = nc.declare_dram_parameter("pmask", [P, 1], F32, isOutput=False)
    out_ext = nc.declare_dram_parameter("out", [EMB, 1], F32, isOutput=True)

    nchunk = len(chunks)
    agh_in = [nc.dram_tensor(f"aghin{c}", [crows[c], EMB], F32)
              for c in range(nchunk)]
    agh_out = [nc.dram_tensor(f"aghout{c}", [B * crows[c], EMB], F32,
                              addr_space="Shared") for c in range(nchunk)]
    stg = [nc.dram_tensor(f"stg{r}", [P * NRTOT, F], F32)
           for r in range(len(runs))]

    rg = [list(range(B))]
    maxW = max(max(NB * (Cb + 1) for (_, NB, Cb) in batches), 1)
    max_nblk = max((c[2] for rc in calls for c in rc), default=1)
    maxgr = max(g_hi - g_lo for (_, _, g_lo, g_hi) in chunks)

    with tile.TileContext(nc) as tc:
        with tc.tile_pool(name="const", bufs=1) as cpool, \
             tc.tile_pool(name="work", bufs=6) as wpool, \
             tc.tile_pool(name="hbuf", bufs=2) as hpool, \
             tc.tile_pool(name="cbuf", bufs=1) as cbpool, \
             tc.tile_pool(name="psum", bufs=2, space="PSUM") as ppool, \
             tc.tile_pool(name="psumt", bufs=1, space="PSUM") as tpool:

            w1t = cpool.tile([F + 1, H1], F32)
            nc.sync.dma_start(out=w1t[:, :], in_=w1e_in[:, :])
            w2t = cpool.tile([H1, EMB], F32)
            nc.sync.dma_start(out=w2t[:, :], in_=w2_in[:, :])
            fct = cpool.tile([EMB + 1, EMB], F32)
            nc.sync.dma_start(out=fct[:, :], in_=fce_in[:, :])
            outt = cpool.tile([EMB + 1, EMB], F32)
            nc.sync.dma_start(out=outt[:, :], in_=oute_in[:, :])
            b2t = cpool.tile([P, 4 * EMB], F32)
            nc.sync.dma_start(out=b2t[:, :], in_=b2b_in[:, :])
            pmt = cpool.tile([P, 1], F32)
            nc.sync.dma_start(out=pmt[:, :], in_=pmask_in[:, :])
            ident = cpool.tile([P, P], F32)
            make_identity(nc, ident[:, :])
            disp = cpool.tile([P, G], F32)
            nc.sync.dma_start(out=disp[:, :], in_=degp_in[:, :])
            nc.scalar.sqrt(out=disp[:, :], in_=disp[:, :])
            nc.vector.reciprocal(out=disp[:, :], in_=disp[:, :])
            ones_col = cpool.tile([P, 1], F32)
            nc.vector.memset(ones_col[:, :], 1.0)
            zrow = cpool.tile([P, F], F32)
            nc.vector.memset(zrow[:, :], 0.0)
            pool_acc = cpool.tile([P, 4 * EMB], F32)
            nc.vector.memset(pool_acc[:, :], 0.0)

            for r in range(len(runs)):
                sv = stg[r][:, :].rearrange("(p n) f -> p n f", n=NRTOT)
                nc.sync.dma_start(out=sv[:, 0, :], in_=zrow[:, :])

            flat_calls = []
            for ri in range(len(runs)):
                nofs = 1
                for (ccx, wi, nblk, num_, i1o) in calls[ri]:
                    flat_calls.append((ccx, ri, wi, nblk, num_, i1o, nofs))
                    nofs += nblk
            flat_calls.sort(key=lambda t: (t[0], t[1], t[2]))

            def emit_stage1(chunk_id):
                for (ccx, ri, wi, nblk, m, i1o, nofs) in flat_calls:
                    if ccx != chunk_id:
                        continue
                    num = nblk * P
                    it = wpool.tile([P, 8 * max_nblk], I16, tag="i1t")
                    nc.sync.dma_start(
                        out=it[:, :num // 16],
                        in_=i1_in[i1o:i1o + P * (num // 16)]
                            .rearrange("(p s) -> p s", p=P))
                    Ts = wpool.tile([P, max_nblk * F], F32, tag="st1")
                    w0 = wi * WIN
                    w1 = min(w0 + WIN, B * crows[ccx])
                    nc.gpsimd.dma_gather(
                        Ts[:, :nblk * F].rearrange("p (n f) -> p n f", f=F),
                        agh_out[ccx][w0:w1, :], it[:, :num // 16],
                        num, m, F, single_packet=False)
                    sv = stg[ri][:, :].rearrange("(p n) f -> p n f", n=NRTOT)
                    nc.sync.dma_start(out=sv[:, nofs:nofs + nblk, :],
                                      in_=Ts[:, :nblk * F])

            # ---------------- conv1 ----------------
            for ci, (blo, bhi, g_lo, g_hi) in enumerate(chunks):
                ngr = g_hi - g_lo
                hbt = hpool.tile([P, maxgr * EMB], F32, tag="hc")
                for bi in range(blo, bhi):
                    g0, NB, Cb = batches[bi]
                    Cb1 = Cb + 1
                    W1b = NB * Cb1
                    A = wpool.tile([P, 4 * (F + 1)], F32, tag="aext")
                    Av = A[:, :NB * (F + 1)].rearrange("p (g f) -> p g f", g=NB)
                    T = wpool.tile([P, maxW * F], F32, tag="gat")
                    nc.sync.dma_start(
                        out=T[:, :W1b * F],
                        in_=g1_in[bofs1[bi] * F:(bofs1[bi] + P * W1b) * F]
                            .rearrange("(p w) -> p w", p=P))
                    Tv = T[:, :W1b * F].rearrange("p (g c f) -> p g c f",
                                                  g=NB, c=Cb1)
                    _segsum(nc, Tv, Cb1)
                    nc.vector.tensor_tensor(
                        out=Av[:, :, 0:F], in0=Tv[:, :, 0, :],
                        in1=disp[:, g0:g0 + NB].to_broadcast([P, NB, F]),
                        op=mybir.AluOpType.mult)
                    nc.vector.memset(Av[:, :, F:F + 1], 1.0)
                    Tp = ppool.tile([F + 1, 4 * P], F32, tag="pt")
                    for gl in range(NB):
                        nc.tensor.transpose(out=Tp[:, gl * P:(gl + 1) * P],
                                            in_=Av[:, gl, :], identity=ident[:, :])
                    aT = wpool.tile([F + 1, 4 * P], F32, tag="aT")
                    nc.scalar.copy(out=aT[:, :NB * P], in_=Tp[:, :NB * P])
                    H1p = ppool.tile([P, 4 * P], F32, tag="h1p")
                    nc.tensor.matmul(H1p[:, :NB * P], w1t[:, :], aT[:, :NB * P],
                                     start=True, stop=True)
                    h1s = wpool.tile([P, 4 * P], F32, tag="h1s")
                    nc.scalar.activation(out=h1s[:, :NB * P], in_=H1p[:, :NB * P],
                                         func=mybir.ActivationFunctionType.Relu)
                    H2p = ppool.tile([P, 4 * EMB], F32, tag="h2p")
                    for gl in range(NB):
                        nc.tensor.matmul(H2p[:, gl * EMB:(gl + 1) * EMB],
                                         h1s[:, gl * P:(gl + 1) * P], w2t[:, :],
                                         start=True, stop=True)
                    hofs = (g0 - g_lo) * EMB
                    Hv = hbt[:, hofs:hofs + NB * EMB].rearrange(
                        "p (g f) -> p g f", g=NB)
                    nc.vector.tensor_tensor(
                        out=Hv,
                        in0=H2p[:, :NB * EMB].rearrange("p (g f) -> p g f", g=NB),
                        in1=disp[:, g0:g0 + NB].to_broadcast([P, NB, EMB]),
                        op=mybir.AluOpType.mult)
                    if g0 + NB == G:
                        nc.vector.tensor_scalar_mul(
                            out=hbt[:, hofs + (NB - 1) * EMB:hofs + NB * EMB],
                            in0=hbt[:, hofs + (NB - 1) * EMB:hofs + NB * EMB],
                            scalar1=pmt[:, 0:1])
                nc.sync.dma_start(
                    out=agh_in[ci][:, :].rearrange("(n p) f -> p n f", p=P),
                    in_=hbt[:, :ngr * EMB])
                nc.gpsimd.collective_compute(
                    "AllGather", mybir.AluOpType.bypass, replica_groups=rg,
                    ins=[agh_in[ci][:, :]], outs=[agh_out[ci][:, :]])

            # ---------------- conv2 stage 1 ----------------
            for chunk_id in range(len(chunks)):
                emit_stage1(chunk_id)

            # ---------------- conv2 stage 2 + pool ----------------
            i2o = 0
            prev_ci = -1
            for (blo2, bhi2, wsum) in s2groups:
                ri = next(r for r, (lo, hi) in enumerate(runs)
                          if lo <= blo2 < hi)
                if wsum > 0:
                    num = P * wsum
                    it = wpool.tile([P, 8 * 32], I16, tag="i2t")
                    nc.sync.dma_start(
                        out=it[:, :num // 16],
                        in_=i2_in[i2o:i2o + P * (num // 16)]
                            .rearrange("(p s) -> p s", p=P))
                    T = wpool.tile([P, 32 * F], F32, tag="gat")
                    nc.gpsimd.dma_gather(
                        T[:, :wsum * F].rearrange("p (n f) -> p n f", f=F),
                        stg[ri][:, :], it[:, :num // 16],
                        num, num, F, single_packet=False)
                    i2o += P * (num // 16)
                wofs = 0
                for bi in range(blo2, bhi2):
                    g0, NB, Cb = batches[bi]
                    W = NB * Cb
                    ci, g_lo_c, g_hi_c = next(
                        (c, gl, gh) for c, (blo, bhi, gl, gh)
                        in enumerate(chunks) if blo <= bi < bhi)
                    X2 = wpool.tile([P, 4 * EMB], F32, tag="x2")
                    X2v = X2[:, :NB * EMB].rearrange("p (g f) -> p g f", g=NB)
                    selfd = agh_in[ci][:, :].rearrange("(n p) f -> p n f", p=P)
                    stt = wpool.tile([P, 4 * EMB], F32, tag="selft")
                    nc.sync.dma_start(
                        out=stt[:, :NB * EMB],
                        in_=selfd[:, g0 - g_lo_c:g0 - g_lo_c + NB, :])
                    selfv = stt[:, :NB * EMB].rearrange("p (g f) -> p g f", g=NB)
                    if Cb > 0:
                        Tv = T[:, wofs * F:(wofs + W) * F].rearrange(
                            "p (g c f) -> p g c f", g=NB, c=Cb)
                        _segsum(nc, Tv, Cb)
                        nc.vector.tensor_tensor(
                            out=X2v, in0=Tv[:, :, 0, :],
                            in1=selfv, op=mybir.AluOpType.add)
                        wofs += W
                    else:
                        nc.vector.tensor_copy(out=X2v, in_=selfv)
                    nc.vector.tensor_tensor(
                        out=X2v, in0=X2v,
                        in1=disp[:, g0:g0 + NB].to_broadcast([P, NB, EMB]),
                        op=mybir.AluOpType.mult)
                    nc.vector.tensor_tensor(
                        out=X2[:, :NB * EMB], in0=X2[:, :NB * EMB],
                        in1=b2t[:, :NB * EMB], op=mybir.AluOpType.add)
                    nc.vector.tensor_scalar_max(out=X2[:, :NB * EMB],
                                                in0=X2[:, :NB * EMB], scalar1=0.0)
                    if g0 + NB == G:
                        nc.vector.tensor_scalar_mul(
                            out=X2[:, (NB - 1) * EMB:NB * EMB],
                            in0=X2[:, (NB - 1) * EMB:NB * EMB],
                            scalar1=pmt[:, 0:1])
                    nc.vector.tensor_tensor(out=pool_acc[:, :NB * EMB],
                                            in0=pool_acc[:, :NB * EMB],
                                            in1=X2[:, :NB * EMB],
                                            op=mybir.AluOpType.add)

            # ---------------- pooled mean + FC head ----------------
            pv = pool_acc[:, :].rearrange("p (q f) -> p q f", q=4)
            nc.vector.tensor_tensor(out=pv[:, 0:2, :], in0=pv[:, 0:2, :],
                                    in1=pv[:, 2:4, :], op=mybir.AluOpType.add)
            nc.vector.tensor_tensor(out=pv[:, 0:1, :], in0=pv[:, 0:1, :],
                                    in1=pv[:, 1:2, :], op=mybir.AluOpType.add)
            Pp = tpool.tile([EMB, 1], F32, tag="tail")
            nc.tensor.matmul(Pp[:, :], pool_acc[:, 0:EMB], ones_col[:, :],
                             start=True, stop=True)
            pl = wpool.tile([EMB + 1, 1], F32, tag="pl")
            nc.scalar.mul(out=pl[0:EMB, :], in_=Pp[:, :], mul=1.0 / n_per)
            nc.vector.memset(pl[EMB:EMB + 1, :], 1.0)
            F1 = tpool.tile([EMB, 1], F32, tag="tail2")
            nc.tensor.matmul(F1[:, :], fct[:, :], pl[:, :], start=True, stop=True)
            f1s = wpool.tile([EMB + 1, 1], F32, tag="f1s")
            nc.vector.tensor_scalar_max(out=f1s[0:EMB, :], in0=F1[:, :], scalar1=0.0)
            nc.vector.memset(f1s[EMB:EMB + 1, :], 1.0)
            F2 = tpool.tile([EMB, 1], F32, tag="tail")
            nc.tensor.matmul(F2[:, :], outt[:, :], f1s[:, :], start=True, stop=True)
            osb = wpool.tile([EMB, 1], F32, tag="osb")
            nc.vector.tensor_copy(out=osb[:, :], in_=F2[:, :])
            nc.sync.dma_start(out=out_ext[:, :], in_=osb[:, :])
    nc.compile()
    return nc


_BUILD_CACHE = {}
LAST_RESULT = None


def kernel(**inputs):
    global LAST_RESULT
    from concourse.bass_utils import run_bass_kernel_spmd
    in_maps, plan = _preprocess(inputs)
    key = (tuple(plan["batches"]), plan["G"], plan["S1"], plan["n_per"],
           tuple(tuple(c[:3] for c in rc) for rc in plan["calls"]))
    if key not in _BUILD_CACHE:
        _BUILD_CACHE[key] = _build(plan)
    nc = _BUILD_CACHE[key]
    res = run_bass_kernel_spmd(nc, in_maps, list(range(B)))
    LAST_RESULT = res
    out = np.stack([res.results[k]["out"][:, 0] for k in range(B)], axis=0)
    return out.astype(np.float32)



# revision 7
# speedup vs baseline: 1.3095x; 1.3095x over previous
"""Trainium2 Bass kernel for CombinedGCN (2x GCNConv + mean-pool + 2 FC).

No-collective design: core k owns graph k (50k nodes), processed in 4
slices of 12500 dst nodes.  For each slice the core computes conv1 (and
h2~ = dis * (relu(conv1) @ W2)) for a local TABLE = {slice-own nodes} u
{sources of the slice's in-edges} (~24.5k nodes, int16-addressable).
conv1's aggregation input is host-pre-gathered (it depends only on x and
edge_index), so duplicating conv1 compute for remote sources removes
every cross-device exchange.  conv2 is then a single local dma_gather
from the slice table into a degree-bucketed segment layout + DVE
segmented sum + self term + scale/bias/relu + mean-pool.

conv1 runs transposed (features on partitions, node-pairs along free):
host emits g1 with both dis scalings folded in, DVE folds edge slots,
and two weight-padded bf16 matmuls (even/odd node parity) compute h1 for
512 nodes per batch with no input-side PE transposes.  h2~ returns to
row layout via PE identity transposes; the even/odd halves land in the
two free-halves of one SBUF tile so each 128-pair chunk writes 256
consecutive table rows with one contiguous DMA.
"""
import sys

import numpy as np

sys.path.insert(0, "/opt/trn_rl_repo")

import ml_dtypes  # noqa: E402

from concourse import bass, bacc, mybir, tile  # noqa: E402
from concourse.masks import make_identity  # noqa: E402

B = 8
N_PER = 50000
NSLICE = 4
SL_N = N_PER // NSLICE          # 12500
F = 64
H1 = 128
EMB = 64
P = 128
F32 = mybir.dt.float32
BF16 = mybir.dt.bfloat16
I16 = mybir.dt.int16
BF = ml_dtypes.bfloat16

NB1 = 8                         # conv1 buckets per batch (kept even)
BUD1 = 96                       # conv1 NB*C budget (tile cols/64)
CALL_COLS = 64                  # conv2 gather-call budget (cols of 128)
G2 = (SL_N + P - 1) // P        # 98 conv2 buckets per slice


def _wrap_idx16(flat):
    """[num] int16 (num % 16 == 0) -> [128, num//16] wrapped + replicated."""
    num = len(flat)
    s = flat.reshape(num // 16, 16).T
    return np.tile(s, (8, 1)).astype(np.int16)


def _run_offsets(sorted_ids):
    """Position of each element within its run of equal sorted_ids."""
    n = len(sorted_ids)
    if n == 0:
        return np.zeros(0, np.int64)
    boundary = np.concatenate(([True], sorted_ids[1:] != sorted_ids[:-1]))
    run_id = np.cumsum(boundary) - 1
    run_start = np.flatnonzero(boundary)
    return np.arange(n) - run_start[run_id]


def _batches_common(Cb, NBmax, budget, even=False, slack=0.13):
    """Waste-bounded greedy batches (g0, NB, C) of consecutive buckets.

    C = max Cb over the batch; NB <= NBmax, NB*max(C,1) <= budget; a
    batch only grows while the padding it adds stays under `slack` of
    its content.  With even=True batches grow in pairs so every batch
    except possibly the last has even NB (keeps g0 even for the
    bucket-pair aligned transpose chunks)."""
    G = len(Cb)
    step = 2 if even else 1
    out = []
    g = 0
    while g < G:
        NB = 1
        C = int(Cb[g])
        sumC = C
        if even and g + 1 < G:
            C = max(C, int(Cb[g + 1]))
            sumC += int(Cb[g + 1])
            NB = 2
        while g + NB + step <= G and NB + step <= NBmax:
            newC = max([C] + [int(Cb[g + NB + i]) for i in range(step)])
            addsum = sum(int(Cb[g + NB + i]) for i in range(step))
            if (NB + step) * max(newC, 1) > budget:
                break
            waste = (NB + step) * newC - (sumC + addsum)
            if waste > max(2, int(slack * (sumC + addsum))):
                break
            C = newC
            sumC += addsum
            NB += step
        out.append((g, NB, C))
        g += NB
    return out


def _plan_and_build(inputs):
    x = np.ascontiguousarray(
        np.asarray(inputs["node_features"], np.float32)).reshape(-1, F)
    ei = np.asarray(inputs["edge_index"]).reshape(2, -1)
    src = ei[0].astype(np.int64)
    dst = ei[1].astype(np.int64)
    N = x.shape[0]
    creal = np.bincount(dst, minlength=N)
    deg = creal + 1
    dis = (1.0 / np.sqrt(deg.astype(np.float64))).astype(np.float32)

    eo = np.argsort(dst, kind="stable")          # edges by dst
    s_s = src[eo]
    starts = np.zeros(N + 1, np.int64)
    starts[1:] = np.cumsum(creal)

    es_o = np.argsort(src, kind="stable")        # edges by src
    s_bysrc = src[es_o]
    d_bysrc = dst[es_o]

    # ------------- pass 1: tables + common plan -------------
    tables = {}
    plan = []
    for s in range(NSLICE):
        Tmax = 0
        for k in range(B):
            lo = k * N_PER + s * SL_N
            e0, e1 = starts[lo], starts[lo + SL_N]
            es = s_s[e0:e1]
            own = np.arange(lo, lo + SL_N)
            rem = np.setdiff1d(np.unique(es), own)
            oo = own[np.lexsort((own, -deg[own]))]
            ro = rem[np.lexsort((rem, -deg[rem]))]
            tbl = np.concatenate([oo, ro])
            tables[(k, s)] = tbl
            Tmax = max(Tmax, len(tbl))
        G1 = (Tmax + P - 1) // P
        assert 1 + G1 * P < 32767
        Cb1 = np.zeros(G1, np.int64)
        Cb2 = np.zeros(G2, np.int64)
        for k in range(B):
            tbl = tables[(k, s)]
            dpad = np.zeros(G1 * P, np.int64)
            dpad[:len(tbl)] = deg[tbl]
            Cb1 = np.maximum(Cb1, dpad.reshape(G1, P).max(axis=1))
            d2 = np.zeros(G2 * P, np.int64)
            d2[:SL_N] = creal[tbl[:SL_N]]
            Cb2 = np.maximum(Cb2, d2.reshape(G2, P).max(axis=1))
        b1 = _batches_common(Cb1, NB1, BUD1, even=True)
        cbase1 = [0]
        for (_, NB, C) in b1:
            cbase1.append(cbase1[-1] + NB * C * 64)
        b2 = _batches_common(Cb2, 8, CALL_COLS)
        cbase2 = [0]
        for (_, NB, C) in b2:
            cbase2.append(cbase2[-1] + NB * C)
        calls = []
        bi = 0
        while bi < len(b2):
            c0 = cbase2[bi]
            hi_b = bi + 1
            while hi_b < len(b2) and cbase2[hi_b + 1] - c0 <= CALL_COLS:
                hi_b += 1
            calls.append((bi, hi_b, c0, cbase2[hi_b]))
            bi = hi_b
        plan.append(dict(Tmax=Tmax, G1=G1, b1=b1, cbase1=cbase1,
                         W1tot=cbase1[-1], b2=b2, cbase2=cbase2,
                         W2tot=cbase2[-1], calls=calls, Trows=1 + G1 * P))

    # ------------- pass 2: per-core arrays -------------
    w1 = np.asarray(inputs["W1"], np.float32)
    w2 = np.asarray(inputs["W2"], np.float32)
    w1e = np.zeros((P, H1), np.float32)
    w1o = np.zeros((P, H1), np.float32)
    w1e[:F] = w1
    w1o[F:] = w1
    b1v = np.asarray(inputs["b1"], np.float32).reshape(H1, 1)
    b2row = np.tile(np.asarray(inputs["b2"], np.float32)[None, :], (P, 1))
    fce = np.concatenate([np.asarray(inputs["fc_w"], np.float32),
                          np.asarray(inputs["fc_b"], np.float32)[None, :]], 0)
    oute = np.concatenate([np.asarray(inputs["out_w"], np.float32),
                           np.asarray(inputs["out_b"], np.float32)[None, :]],
                          0)
    pm2 = np.zeros((P, 1), np.float32)
    pm2[:SL_N - (G2 - 1) * P] = 1.0
    xs = x * dis[:, None]

    in_maps = []
    for k in range(B):
        g1s, i2s, disTs, disp2s = [], [], [], []
        for s in range(NSLICE):
            pl = plan[s]
            G1, b1b, cbase1 = pl["G1"], pl["b1"], pl["cbase1"]
            tbl = tables[(k, s)]
            T = len(tbl)
            lo = k * N_PER + s * SL_N
            tpos = np.full(N, -1, np.int64)
            tpos[tbl] = np.arange(T)
            C_of_g = np.zeros(G1, np.int64)
            base_of_g = np.zeros(G1, np.int64)
            goff_of_g = np.zeros(G1, np.int64)
            for bi, (g0, NB, C) in enumerate(b1b):
                C_of_g[g0:g0 + NB] = C
                base_of_g[g0:g0 + NB] = cbase1[bi]
                goff_of_g[g0:g0 + NB] = np.arange(NB)
            # ---- g1: in-edges of table nodes + self slots ----
            vsel = tpos[d_bysrc] >= 0
            eu = s_bysrc[vsel]
            ev = d_bysrc[vsel]
            q = tpos[ev]
            ord2 = np.lexsort((np.arange(len(ev)), q))
            eu, q = eu[ord2], q[ord2]
            ev = ev[ord2]
            cc = _run_offsets(q)
            gq = q // P
            lq = q % P
            # col = base + (goff*C + c)*64 + pair ; partition = 64*par + f
            colE = (base_of_g[gq] + (goff_of_g[gq] * C_of_g[gq] + cc) * 64
                    + lq // 2)
            W1tot = pl["W1tot"]
            g1v = np.zeros((2, F, W1tot), np.float32)
            g1v[lq % 2, :, colE] = xs[eu] * dis[ev][:, None]
            qq = np.arange(T)
            gs = qq // P
            ls = qq % P
            colS = (base_of_g[gs]
                    + (goff_of_g[gs] * C_of_g[gs] + creal[tbl]) * 64
                    + ls // 2)
            g1v[ls % 2, :, colS] = xs[tbl] * dis[tbl][:, None]
            g1s.append(g1v.reshape(P, W1tot).astype(BF))
            # ---- disT [P, 2*npg]: col pg*2+e -> dis(bucket 2pg+p//64,
            #      lane 2*(p%64)+e) ----
            npg = (G1 + 1) // 2
            dpad = np.ones(G1 * P, np.float32)
            dpad[:T] = dis[tbl]
            dpad = dpad.reshape(G1, P)
            dTc = np.ones((P, 2 * npg), np.float32)
            pvec = np.arange(P)
            for pg in range(npg):
                gsel = np.clip(2 * pg + pvec // 64, 0, G1 - 1)
                lsel = 2 * (pvec % 64)
                dTc[:, 2 * pg] = dpad[gsel, lsel]
                dTc[:, 2 * pg + 1] = dpad[gsel, lsel + 1]
            disTs.append(dTc)
            # ---- conv2: i2 + disp2 ----
            b2b, cbase2 = pl["b2"], pl["cbase2"]
            e0, e1 = starts[lo], starts[lo + SL_N]
            es2 = s_s[e0:e1]
            d2list = tpos[np.repeat(np.arange(lo, lo + SL_N),
                                    creal[lo:lo + SL_N])]
            ord3 = np.lexsort((np.arange(len(es2)), d2list))
            es2s = es2[ord3]
            q2s = d2list[ord3]
            cc2 = _run_offsets(q2s)
            g2v = q2s // P
            p2v = q2s % P
            C2_of_g = np.zeros(G2, np.int64)
            base2_of_g = np.zeros(G2, np.int64)
            goff2_of_g = np.zeros(G2, np.int64)
            for bi, (g0, NB, C) in enumerate(b2b):
                C2_of_g[g0:g0 + NB] = C
                base2_of_g[g0:g0 + NB] = cbase2[bi]
                goff2_of_g[g0:g0 + NB] = np.arange(NB)
            i2_flat = np.zeros(max(pl["W2tot"], 1) * P, np.int64)
            cols2 = (base2_of_g[g2v] + goff2_of_g[g2v] * C2_of_g[g2v]
                     + cc2)
            i2_flat[cols2 * P + p2v] = 1 + tpos[es2s]
            i2s.append(i2_flat)
            d2pad = np.ones(G2 * P, np.float32)
            d2pad[:SL_N] = dis[tbl[:SL_N]]
            disp2s.append(d2pad.reshape(G2, P).T.copy())
        i2w = []
        for s in range(NSLICE):
            for (blo, bhi, c0, c1) in plan[s]["calls"]:
                if c1 > c0:
                    seg = i2s[s][c0 * P:c1 * P].astype(np.int16)
                    i2w.append(_wrap_idx16(seg).reshape(-1))
        in_maps.append({
            "g1": np.concatenate(g1s, axis=1),
            "i2": (np.concatenate(i2w) if i2w
                   else np.zeros(16, np.int16)),
            "disT": np.concatenate(disTs, axis=1),
            "disp2": np.concatenate(disp2s, axis=1),
            "w1e": w1e.astype(BF), "w1o": w1o.astype(BF),
            "w2": w2.astype(BF), "b1v": b1v, "b2row": b2row,
            "fce": fce, "oute": oute, "pm2": pm2,
        })
    shp = dict(g1_w=in_maps[0]["g1"].shape[1],
               i2_len=len(in_maps[0]["i2"]),
               disT_w=in_maps[0]["disT"].shape[1],
               disp2_w=G2 * NSLICE)
    for m in in_maps:
        assert m["g1"].shape[1] == shp["g1_w"]
        assert len(m["i2"]) == shp["i2_len"]
    return plan, in_maps, shp


def _fold4(nc, Tv, C):
    """Fold [p, nb, C, x] into C-index 0 (axis 2)."""
    cc = C
    h = 1 << (cc.bit_length() - 1)
    if h < cc:
        nc.vector.tensor_tensor(
            out=Tv[:, :, 0:cc - h, :], in0=Tv[:, :, 0:cc - h, :],
            in1=Tv[:, :, h:cc, :], op=mybir.AluOpType.add)
    cc = h
    while cc > 1:
        cc //= 2
        nc.vector.tensor_tensor(
            out=Tv[:, :, 0:cc, :], in0=Tv[:, :, 0:cc, :],
            in1=Tv[:, :, cc:2 * cc, :], op=mybir.AluOpType.add)


def _build(plan, shp):
    nc = bacc.Bacc("TRN2", target_bir_lowering=False, debug=False,
                   num_devices=B)
    g1_in = nc.declare_dram_parameter("g1", [P, shp["g1_w"]], BF16,
                                      isOutput=False)
    i2_in = nc.declare_dram_parameter("i2", [max(shp["i2_len"], 16)], I16,
                                      isOutput=False)
    disT_in = nc.declare_dram_parameter("disT", [P, shp["disT_w"]], F32,
                                        isOutput=False)
    disp2_in = nc.declare_dram_parameter("disp2", [P, shp["disp2_w"]], F32,
                                         isOutput=False)
    w1e_in = nc.declare_dram_parameter("w1e", [P, H1], BF16, isOutput=False)
    w1o_in = nc.declare_dram_parameter("w1o", [P, H1], BF16, isOutput=False)
    w2_in = nc.declare_dram_parameter("w2", [H1, EMB], BF16, isOutput=False)
    b1_in = nc.declare_dram_parameter("b1v", [H1, 1], F32, isOutput=False)
    b2_in = nc.declare_dram_parameter("b2row", [P, EMB], F32, isOutput=False)
    fce_in = nc.declare_dram_parameter("fce", [EMB + 1, EMB], F32,
                                       isOutput=False)
    oute_in = nc.declare_dram_parameter("oute", [EMB + 1, EMB], F32,
                                        isOutput=False)
    pm2_in = nc.declare_dram_parameter("pm2", [P, 1], F32, isOutput=False)
    out_ext = nc.declare_dram_parameter("out", [EMB, 1], F32, isOutput=True)

    tbls = [nc.dram_tensor(f"tbl{s}", [plan[s]["Trows"], F], F32)
            for s in range(NSLICE)]

    with tile.TileContext(nc) as tc:
        with tc.tile_pool(name="const", bufs=1) as cpool, \
             tc.tile_pool(name="g1b", bufs=3) as gpool, \
             tc.tile_pool(name="work", bufs=4) as wpool, \
             tc.tile_pool(name="gat", bufs=2) as tgpool, \
             tc.tile_pool(name="x2", bufs=2) as xpool, \
             tc.tile_pool(name="psA", bufs=2, space="PSUM") as psA, \
             tc.tile_pool(name="psB", bufs=1, space="PSUM") as psB, \
             tc.tile_pool(name="psT", bufs=2, space="PSUM") as psT:

            w1et = cpool.tile([P, H1], BF16)
            nc.sync.dma_start(out=w1et[:, :], in_=w1e_in[:, :])
            w1ot = cpool.tile([P, H1], BF16)
            nc.sync.dma_start(out=w1ot[:, :], in_=w1o_in[:, :])
            w2t = cpool.tile([H1, EMB], BF16)
            nc.sync.dma_start(out=w2t[:, :], in_=w2_in[:, :])
            b1t = cpool.tile([H1, 1], F32)
            nc.sync.dma_start(out=b1t[:, :], in_=b1_in[:, :])
            b2t = cpool.tile([P, EMB], F32)
            nc.sync.dma_start(out=b2t[:, :], in_=b2_in[:, :])
            fct = cpool.tile([EMB + 1, EMB], F32)
            nc.sync.dma_start(out=fct[:, :], in_=fce_in[:, :])
            outt = cpool.tile([EMB + 1, EMB], F32)
            nc.sync.dma_start(out=outt[:, :], in_=oute_in[:, :])
            pmt = cpool.tile([P, 1], F32)
            nc.sync.dma_start(out=pmt[:, :], in_=pm2_in[:, :])
            disTt = cpool.tile([P, shp["disT_w"]], F32)
            nc.sync.dma_start(out=disTt[:, :], in_=disT_in[:, :])
            disp2t = cpool.tile([P, shp["disp2_w"]], F32)
            nc.sync.dma_start(out=disp2t[:, :], in_=disp2_in[:, :])
            ident = cpool.tile([P, P], F32)
            make_identity(nc, ident[:, :])
            ones_col = cpool.tile([P, 1], F32)
            nc.vector.memset(ones_col[:, :], 1.0)
            zrow = cpool.tile([1, F], F32)
            nc.vector.memset(zrow[:, :], 0.0)
            pool_acc = cpool.tile([P, EMB], F32)
            nc.vector.memset(pool_acc[:, :], 0.0)

            for s in range(NSLICE):
                nc.sync.dma_start(out=tbls[s][0:1, :], in_=zrow[:, :])

            g1_off = 0
            i2_off = 0
            disT_off = 0
            for s in range(NSLICE):
                pl = plan[s]
                G1, b1b, cbase1 = pl["G1"], pl["b1"], pl["cbase1"]
                tbl = tbls[s]
                # ---------------- conv1 (transposed) ----------------
                for bi, (g0, NB, C) in enumerate(b1b):
                    wcols = NB * C * 64
                    ncols = NB * 64
                    gt = gpool.tile([P, BUD1 * 64], BF16, tag="g1t")
                    nc.sync.dma_start(
                        out=gt[:, :wcols],
                        in_=g1_in[:, g1_off + cbase1[bi]:
                                  g1_off + cbase1[bi] + wcols])
                    Tv = gt[:, :wcols].rearrange(
                        "p (nb c pr) -> p nb c pr", nb=NB, c=C)
                    if C > 1:
                        _fold4(nc, Tv, C)
                    rhs = gt[:, :wcols].rearrange(
                        "p (nb c pr) -> p nb c pr", nb=NB, c=C)[:, :, 0, :]
                    h1 = psA.tile([P, 1024], F32, tag="h1")
                    nc.tensor.matmul(h1[:, :ncols], w1et[:, :], rhs,
                                     start=True, stop=True)
                    nc.tensor.matmul(h1[:, 512:512 + ncols], w1ot[:, :],
                                     rhs, start=True, stop=True)
                    h1s = wpool.tile([H1, 1024], BF16, tag="h1s")
                    nc.scalar.activation(
                        out=h1s[:, :ncols], in_=h1[:, :ncols],
                        func=mybir.ActivationFunctionType.Relu,
                        bias=b1t[:, 0:1])
                    nc.scalar.activation(
                        out=h1s[:, 512:512 + ncols],
                        in_=h1[:, 512:512 + ncols],
                        func=mybir.ActivationFunctionType.Relu,
                        bias=b1t[:, 0:1])
                    h2T = psB.tile([EMB, 1024], F32, tag="h2T")
                    nc.tensor.matmul(h2T[:, :ncols], w2t[:, :],
                                     h1s[:, :ncols], start=True, stop=True)
                    nc.tensor.matmul(h2T[:, 512:512 + ncols], w2t[:, :],
                                     h1s[:, 512:512 + ncols],
                                     start=True, stop=True)
                    h2s = wpool.tile([EMB, 1024], F32, tag="h2s")
                    nc.scalar.copy(out=h2s[:, :ncols], in_=h2T[:, :ncols])
                    nc.scalar.copy(out=h2s[:, 512:512 + ncols],
                                   in_=h2T[:, 512:512 + ncols])
                    nch = (NB + 1) // 2
                    for m in range(nch):
                        c0 = m * P
                        mm = min(P, ncols - c0)
                        gA = g0 + 2 * m
                        tp = psT.tile([P, 2 * EMB], F32, tag="tp")
                        nc.tensor.transpose(
                            out=tp[:mm, 0:EMB], in_=h2s[:, c0:c0 + mm],
                            identity=ident[0:EMB, 0:EMB])
                        nc.tensor.transpose(
                            out=tp[:mm, EMB:2 * EMB],
                            in_=h2s[:, 512 + c0:512 + c0 + mm],
                            identity=ident[0:EMB, 0:EMB])
                        rows = wpool.tile([P, 2 * EMB], F32, tag="rows")
                        ci = disT_off + (gA // 2) * 2
                        nc.vector.tensor_tensor(
                            out=rows[:mm, :].rearrange(
                                "p (e f) -> p e f", e=2),
                            in0=tp[:mm, :].rearrange(
                                "p (e f) -> p e f", e=2),
                            in1=disTt[:mm, ci:ci + 2].to_broadcast(
                                [mm, 2, EMB]),
                            op=mybir.AluOpType.mult)
                        r0 = 1 + gA * P
                        nc.sync.dma_start(
                            out=tbl[r0:r0 + 2 * mm, :].rearrange(
                                "(j two) f -> j (two f)", two=2),
                            in_=rows[:mm, :])
                # ---------------- conv2 ----------------
                cbase2, b2b = pl["cbase2"], pl["b2"]
                X2 = xpool.tile([P, G2 * EMB], F32, tag="x2")
                for (blo, bhi, c0, c1) in pl["calls"]:
                    Tg = None
                    if c1 > c0:
                        num = (c1 - c0) * P
                        it = wpool.tile([P, CALL_COLS * 8], I16, tag="i2t")
                        nc.sync.dma_start(
                            out=it[:, :num // 16],
                            in_=i2_in[i2_off:i2_off + P * (num // 16)]
                                .rearrange("(p s) -> p s", p=P))
                        i2_off += P * (num // 16)
                        Tg = tgpool.tile([P, CALL_COLS * F], F32, tag="gat")
                        nc.gpsimd.dma_gather(
                            Tg[:, :(c1 - c0) * F].rearrange(
                                "p (n f) -> p n f", f=F),
                            tbl[:, :], it[:, :num // 16],
                            num, num, F, single_packet=False)
                    for bi in range(blo, bhi):
                        g0, NB, C = b2b[bi]
                        selfv = tbl[1 + g0 * P:1 + (g0 + NB) * P, :]\
                            .rearrange("(n p) f -> p n f", p=P)
                        st = wpool.tile([P, 8 * EMB], F32, tag="selft")
                        nc.sync.dma_start(out=st[:, :NB * EMB], in_=selfv)
                        xv = X2[:, g0 * EMB:(g0 + NB) * EMB].rearrange(
                            "p (g f) -> p g f", g=NB)
                        if C > 0:
                            off = cbase2[bi] - c0
                            W = NB * C
                            Tv = Tg[:, off * F:(off + W) * F].rearrange(
                                "p (g c f) -> p g c f", g=NB, c=C)
                            if C > 1:
                                _fold4(nc, Tv, C)
                            nc.vector.tensor_tensor(
                                out=xv, in0=Tv[:, :, 0, :],
                                in1=st[:, :NB * EMB].rearrange(
                                    "p (g f) -> p g f", g=NB),
                                op=mybir.AluOpType.add)
                        else:
                            nc.vector.tensor_copy(
                                out=xv, in_=st[:, :NB * EMB])
                # slice-level: scale, bias, relu(ACT), mask, pool
                xg = X2[:, :].rearrange("p (g f) -> p g f", g=G2)
                nc.vector.tensor_tensor(
                    out=xg, in0=xg,
                    in1=disp2t[:, s * G2:(s + 1) * G2].to_broadcast(
                        [P, G2, EMB]),
                    op=mybir.AluOpType.mult)
                nc.vector.tensor_tensor(
                    out=xg, in0=xg,
                    in1=b2t[:, :].to_broadcast([P, EMB, G2]).rearrange(
                        "p f g -> p g f"),
                    op=mybir.AluOpType.add)
                nc.scalar.activation(
                    out=X2[:, :], in_=X2[:, :],
                    func=mybir.ActivationFunctionType.Relu)
                nc.vector.tensor_scalar_mul(
                    out=X2[:, (G2 - 1) * EMB:G2 * EMB],
                    in0=X2[:, (G2 - 1) * EMB:G2 * EMB],
                    scalar1=pmt[:, 0:1])
                cc = G2
                h = 1 << (cc.bit_length() - 1)
                xf = X2[:, :].rearrange("p (g f) -> p g f", g=G2)
                if h < cc:
                    nc.vector.tensor_tensor(
                        out=xf[:, 0:cc - h, :], in0=xf[:, 0:cc - h, :],
                        in1=xf[:, h:cc, :], op=mybir.AluOpType.add)
                cc = h
                while cc > 1:
                    cc //= 2
                    nc.vector.tensor_tensor(
                        out=xf[:, 0:cc, :], in0=xf[:, 0:cc, :],
                        in1=xf[:, cc:2 * cc, :], op=mybir.AluOpType.add)
                nc.vector.tensor_tensor(
                    out=pool_acc[:, :], in0=pool_acc[:, :],
                    in1=X2[:, 0:EMB], op=mybir.AluOpType.add)
                g1_off += pl["W1tot"]
                disT_off += 2 * ((G1 + 1) // 2)

            # ---------------- pooled mean + FC head ----------------
            Pp = psT.tile([EMB, 1], F32, tag="tp")
            nc.tensor.matmul(Pp[:, :], pool_acc[:, 0:EMB], ones_col[:, :],
                             start=True, stop=True)
            ple = wpool.tile([EMB + 1, 1], F32, tag="pl")
            nc.scalar.mul(out=ple[0:EMB, :], in_=Pp[:, :], mul=1.0 / N_PER)
            nc.vector.memset(ple[EMB:EMB + 1, :], 1.0)
            F1 = psT.tile([EMB, 1], F32, tag="tp")
            nc.tensor.matmul(F1[:, :], fct[:, :], ple[:, :],
                             start=True, stop=True)
            f1s = wpool.tile([EMB + 1, 1], F32, tag="f1s")
            nc.vector.tensor_scalar_max(out=f1s[0:EMB, :], in0=F1[:, :],
                                        scalar1=0.0)
            nc.vector.memset(f1s[EMB:EMB + 1, :], 1.0)
            F2 = psT.tile([EMB, 1], F32, tag="tp")
            nc.tensor.matmul(F2[:, :], outt[:, :], f1s[:, :],
                             start=True, stop=True)
            osb = wpool.tile([EMB, 1], F32, tag="osb")
            nc.vector.tensor_copy(out=osb[:, :], in_=F2[:, :])
            nc.sync.dma_start(out=out_ext[:, :], in_=osb[:, :])
    nc.compile()
    return nc


_BUILD_CACHE = {}
LAST_RESULT = None


def kernel(**inputs):
    global LAST_RESULT
    from concourse.bass_utils import run_bass_kernel_spmd
    plan, in_maps, shp = _plan_and_build(inputs)
    key = tuple((tuple(pl["b1"]), tuple(pl["b2"]), pl["Tmax"])
                for pl in plan)
    if key not in _BUILD_CACHE:
        _BUILD_CACHE[key] = _build(plan, shp)
    nc = _BUILD_CACHE[key]
    res = run_bass_kernel_spmd(nc, in_maps, list(range(B)))
    LAST_RESULT = res
    out = np.stack([res.results[k]["out"][:, 0] for k in range(B)], axis=0)
    return out.astype(np.float32)
